# revision 37
# baseline (speedup 1.0000x reference)
"""EquivariantCrystalGCN forward on 8 TRN2 NeuronCores (Bass/Tile).

Sharding: nodes and their incident (source-side) edges are split across the
8 cores by contiguous node range; MLP weights are replicated. Message
scatter is local to each core; h is exchanged between layers with an
in-kernel AllGather; only the pooled [64,128] matrix is AllReduced at the
end (data/graph parallel).

Structure (v3): edges are grouped per 128-node block, bank-major. All
layer-invariant per-edge operands (edge_attr feature-major, scatter one-hot
qs, o/d rows) are packed per block in DRAM and loaded with two DMAs per
block; h[col] gathers run at (block, bank-run) granularity, capped at 896
indices per call (SWDGE ring limit). Scatter matmuls accumulate in PSUM per
joint into an f32 SBUF block accumulator. The inter-layer AllGather is
asynchronous: only the next layer's gather instructions wait on its
completion semaphore (attached post-schedule), so weight/operand loads and
the h@W1a precompute overlap the collective.

kernel(**inputs) takes the FULL unsharded inputs (np arrays, dtypes as in
setup_inputs) and returns the FULL [64, 128] float32 output.
"""
import sys
sys.path.insert(0, "/opt/trn_rl_repo")

import json
import numpy as np
import ml_dtypes

import jax
from jax.sharding import Mesh, PartitionSpec
from jax.experimental.shard_map import shard_map

import concourse.bass as bass
import concourse.bacc as bacc
import concourse.mybir as mybir
import concourse.tile as tile
from concourse import bass2jax
from concourse.bass2jax import (
    _bass_exec_p,
    partition_id_tensor,
    install_neuronx_cc_hook,
)
from concourse.masks import make_identity

# ---------------------------------------------------------------- constants
N, E, H, R, L, G = 50000, 800000, 128, 128, 3, 64
CUTOFF = 5.0
NC = 8                      # cores
NPC = N // NC               # nodes per core (6250)
NPAD = 6272                 # padded nodes per core (49 * 128)
NB = NPAD // 128            # node blocks per core (49)
NTOT = NC * NPAD            # padded global nodes (50176)
BANK_OFF = 17408            # bank-B table view offset (idx fits int16)
JW = 4                      # subtiles per joint tile (512 edges)

f32 = mybir.dt.float32
bf16 = mybir.dt.bfloat16
i16 = mybir.dt.int16

bf = ml_dtypes.bfloat16
import os as _os
ABLATE = set(filter(None, _os.environ.get("KERNEL_ABLATE", "").split(",")))

# ---------------------------------------------------------------- birfix
# This container's walrus accepts at most ONE sync wait per instruction but
# Tile emits several; split extras into standalone EventSemaphore insts.
def _legalize_multiwaits(bir_json: bytes):
    d = json.loads(bir_json)
    n = 0
    for fn in d.get("functions", []):
        for bb in fn.get("blocks", []):
            out = []
            for ins in bb.get("instructions", []):
                si = ins.get("sync_info")
                waits = (si or {}).get("on_wait") or []
                if len(waits) > 1:
                    for k, w in enumerate(waits[:-1]):
                        out.append({
                            "debug": ins.get("debug", 0),
                            "engine": ins["engine"],
                            "ins": [], "outs": [],
                            "name": f"{ins['name']}_xw{k}",
                            "opcode": "EventSemaphore",
                            "sync_info": {"on_update": [], "on_wait": [w]},
                        })
                        n += 1
                    si["on_wait"] = waits[-1:]
                out.append(ins)
            bb["instructions"] = out
    return json.dumps(d).encode(), n


def _install_birfix():
    if getattr(bass.Bass, "_birfix_installed", False):
        return
    orig = bass.Bass.to_json_bytes

    def patched(self, *a, **k):
        raw = orig(self, *a, **k)
        fixed, _ = _legalize_multiwaits(raw)
        return fixed

    bass.Bass.to_json_bytes = patched
    bass.Bass._birfix_installed = True


# ---------------------------------------------------------------- host prep
def _pack_idx16(vals):
    """Pack per-subtile col indices [S, 128] int16 into the dma_gather idx
    layout: [128, S*8] with item i of subtile s at [i%16, s*8 + i//16],
    replicated across the 8 gpsimd cores (partition groups of 16)."""
    S = vals.shape[0]
    v = vals.reshape(S, 8, 16)          # item i = c*16 + p  ->  [s, c, p]
    out = v.transpose(2, 0, 1).reshape(16, S * 8)   # [p, (s, c)]
    return np.tile(out, (8, 1))         # replicate to 128 partitions


def _preprocess(x, edge_index, edge_weight, edge_attr, batch,
                emb, ew1, eb1, ew2, eb2, nw1, nb1, nw2, nb2, linw, linb):
    x = np.asarray(x)
    edge_index = np.asarray(edge_index)
    edge_weight = np.asarray(edge_weight, np.float32)
    edge_attr = np.asarray(edge_attr, np.float32)
    batch = np.asarray(batch)

    h0 = np.asarray(emb, np.float32)[x]                    # [N, H]
    row = edge_index[0].astype(np.int64)
    col = edge_index[1].astype(np.int64)
    d_raw = (edge_weight / CUTOFF).astype(np.float32)

    core = row // NPC                                       # [E]
    rl = (row % NPC).astype(np.int64)                       # row local
    blk = rl // 128
    col_pad = (col // NPC) * NPAD + (col % NPC)             # padded global col
    bank = (col_pad >= 32768).astype(np.int64)

    # global sort by (core, block, bank, row-local)
    order = np.lexsort((rl, bank, blk, core))
    core_s, blk_s, bank_s, rl_s = core[order], blk[order], bank[order], rl[order]
    colp_s = col_pad[order]

    # group key per (core, block, bank)
    gkey = (core_s * NB + blk_s) * 2 + bank_s
    counts = np.bincount(gkey, minlength=NC * NB * 2).reshape(NC, NB, 2)

    # shared structure: subtiles per (block, bank) = max over cores
    S_bk = np.ceil(counts.max(axis=0) / 128).astype(np.int64)   # [NB, 2]
    S_b = S_bk.sum(axis=1)                                       # [NB]
    S_tot = int(S_b.sum())
    E_struct = S_tot * 128

    # structure offsets: subtile slot base per (block, bank); bank-major
    # within each block (bank0 subtiles then bank1 subtiles).
    sub_base = np.zeros((NB, 2), np.int64)
    acc = 0
    blk_sub0 = np.zeros(NB, np.int64)
    for b in range(NB):
        blk_sub0[b] = acc
        sub_base[b, 0] = acc
        acc += int(S_bk[b, 0])
        sub_base[b, 1] = acc
        acc += int(S_bk[b, 1])

    # joints: per block, consecutive groups of <= JW subtiles (may span
    # banks — the gathers write hcol at subtile granularity independently).
    joints = []            # (block, sub0, nsub)
    for b in range(NB):
        nb_ = int(S_b[b])
        o = 0
        while o < nb_:
            w = min(JW, nb_ - o)
            joints.append((b, int(blk_sub0[b]) + o, w))
            o += w
    T_joint = len(joints)

    # per-edge destination slot (per core): rank within its (c,b,k) group
    gstart = np.zeros(NC * NB * 2 + 1, np.int64)
    np.cumsum(np.bincount(gkey, minlength=NC * NB * 2), out=gstart[1:])
    rank = np.arange(len(order)) - gstart[gkey]
    slot = sub_base[blk_s, bank_s] * 128 + rank             # within-core slot

    # block column extents in the packed layout: [ea | qs] per block
    blk_cols = S_b * 128
    blk_off = np.zeros(NB + 1, np.int64)
    np.cumsum(2 * blk_cols, out=blk_off[1:])
    PACK_COLS = int(blk_off[-1])                            # 2 * E_struct

    per_core = []

    # weights packed (shared across cores)
    ew1 = np.asarray(ew1, np.float32)
    w1a = ew1[:, 0:128, :].astype(bf)                        # [L,128,128]
    w1b = ew1[:, 128:256, :].astype(bf)
    w1c = ew1[:, 256:384, :].astype(bf)
    w1d = ew1[:, 384:385, :].astype(bf)                      # [L,1,128]
    eb1c = np.asarray(eb1, np.float32)[:, :, None]           # [L,128,1]
    ew2b = np.asarray(ew2, np.float32).astype(bf)            # [L,128,128]
    # eb2 tiled 4x along free dim for the merged bias matmul
    eb2_t4 = np.tile(np.asarray(eb2, np.float32)[:, None, :], (1, 1, JW)) \
        .reshape(L, 1, JW * H).astype(bf)                    # [L,1,512]
    nw1b_ = np.asarray(nw1, np.float32).astype(bf)           # [L,256,128]
    nb1c = np.asarray(nb1, np.float32)[:, :, None]           # [L,128,1]
    nw2b_ = np.asarray(nw2, np.float32).astype(bf)           # [L,128,128]
    nb2c = np.asarray(nb2, np.float32)[:, :, None]           # [L,128,1]
    linw_f = np.asarray(linw, np.float32)
    linb_c = np.asarray(linb, np.float32)[:, None]           # [128,1]

    cnt = np.bincount(np.asarray(batch), minlength=G).astype(np.float32)
    cnt_inv = (1.0 / np.maximum(cnt, 1.0))[:, None]          # [64,1]

    ones_row = np.ones((1, 512), bf)

    # padded full h0 (global, bf16) — same for all cores
    h0_pad = np.zeros((NTOT, H), np.float32)
    for c in range(NC):
        h0_pad[c * NPAD:c * NPAD + NPC] = h0[c * NPC:(c + 1) * NPC]
    h0_full = h0_pad.astype(bf)

    iota128 = np.arange(128)

    batch_np = np.asarray(batch)
    for c in range(NC):
        m = core_s == c
        sl = slot[m]
        # per-core padded per-slot arrays
        o_pad = np.full(E_struct, -1.0, np.float32)
        o_pad[sl] = (rl_s[m] - blk_s[m] * 128).astype(np.float32)
        d_pad = np.zeros(E_struct, np.float32)
        d_pad[sl] = d_raw[order[m]]
        idxv = np.zeros(E_struct, np.int16)
        iv = colp_s[m] - bank_s[m] * BANK_OFF
        assert iv.min() >= 0 and iv.max() < 32768
        idxv[sl] = iv.astype(np.int16)

        ea_pad = np.zeros((E_struct, R), bf)
        ea_pad[sl] = edge_attr[order[m]].astype(bf)
        ea_all = np.ascontiguousarray(ea_pad.T)              # [128, E_struct]

        # scatter one-hot per subtile: qs[p, s*128 + f] = (o[s*128+p] == f)
        o_mat = o_pad.reshape(S_tot, 128)                    # [s, p]
        qs_all = (o_mat[:, :, None] == iota128[None, None, :])  # [s, p, f]
        qs_all = np.ascontiguousarray(
            qs_all.transpose(1, 0, 2).reshape(128, S_tot * 128)).astype(bf)

        # packed per-block constants: [ea_b | qs_b] contiguous per block
        pack = np.empty((128, PACK_COLS), bf)
        for b in range(NB):
            c0 = int(blk_off[b])
            s0, s1 = int(blk_sub0[b]), int(blk_sub0[b] + S_b[b])
            w_ = int(blk_cols[b])
            pack[:, c0:c0 + w_] = ea_all[:, s0 * 128:s1 * 128]
            pack[:, c0 + w_:c0 + 2 * w_] = qs_all[:, s0 * 128:s1 * 128]

        # od packed on one partition row: per block [o_b | d_b]
        odpack = np.empty((1, 2 * E_struct), bf)
        for b in range(NB):
            c0 = 2 * int(blk_sub0[b]) * 128
            w_ = int(blk_cols[b])
            s0, s1 = int(blk_sub0[b]) * 128, int(blk_sub0[b] + S_b[b]) * 128
            odpack[0, c0:c0 + w_] = o_pad[s0:s1].astype(bf)
            odpack[0, c0 + w_:c0 + 2 * w_] = d_pad[s0:s1].astype(bf)
        idx_all = _pack_idx16(idxv.reshape(S_tot, 128))      # [128, S_tot*8]

        # h0 shard, feature-major fp32 [128, NPAD]
        h0T = np.zeros((H, NPAD), np.float32)
        h0T[:, :NPC] = h0[c * NPC:(c + 1) * NPC].T

        # batch one-hot per block [128, NB*64]
        bq = np.zeros((NPAD, G), np.float32)
        bl = batch_np[c * NPC:(c + 1) * NPC]
        bq[np.arange(NPC), bl] = 1.0
        Bq = np.ascontiguousarray(
            bq.reshape(NB, 128, G).transpose(1, 0, 2).reshape(128, NB * G)
        ).astype(bf)

        per_core.append({
            "pack": pack, "odpack": odpack, "idx_all": idx_all,
            "iota_col": np.arange(128, dtype=np.float32)[:, None],
            "h0T": h0T, "h0_full": h0_full, "Bq": Bq,
            "w1a": w1a, "w1b": w1b, "w1c": w1c, "w1d": w1d, "eb1": eb1c,
            "ew2": ew2b, "eb2t4": eb2_t4, "nw1": nw1b_, "nb1": nb1c,
            "nw2": nw2b_, "nb2": nb2c, "linw": linw_f, "linb": linb_c,
            "cnt_inv": cnt_inv, "ones": ones_row,
        })

    struct = {
        "eb2_zero": bool(np.abs(np.asarray(eb2)).max() == 0),
        "S_tot": S_tot, "E_struct": E_struct, "joints": joints,
        "S_bk": S_bk, "S_b": S_b, "blk_off": blk_off,
        "blk_sub0": blk_sub0, "T_joint": T_joint, "PACK_COLS": PACK_COLS,
    }
    return per_core, struct


# ---------------------------------------------------------------- program
def _build_program(struct):
    S_tot = struct["S_tot"]
    E_struct = struct["E_struct"]
    joints = struct["joints"]
    S_bk = struct["S_bk"]
    S_b = struct["S_b"]
    blk_off = struct["blk_off"]
    blk_sub0 = struct["blk_sub0"]
    PACK_COLS = struct["PACK_COLS"]
    EB2_ZERO = struct["eb2_zero"]
    S_MAX = int(S_b.max())

    nc = bacc.Bacc("TRN2", num_swdge_queues=4, dynamic_dma_scratch_size=32768)

    # ---- I/O
    pack_in = nc.dram_tensor("pack", [128, PACK_COLS], bf16, kind="ExternalInput")
    od_in = nc.dram_tensor("odpack", [1, 2 * E_struct], bf16, kind="ExternalInput")
    iota_in = nc.dram_tensor("iota_col", [128, 1], f32, kind="ExternalInput")
    idx_all = nc.dram_tensor("idx_all", [128, S_tot * 8], i16, kind="ExternalInput")
    h0T_in = nc.dram_tensor("h0T", [128, NPAD], f32, kind="ExternalInput")
    h0_full = nc.dram_tensor("h0_full", [NTOT, 128], bf16, kind="ExternalInput")
    Bq_in = nc.dram_tensor("Bq", [128, NB * G], bf16, kind="ExternalInput")
    w1a_in = nc.dram_tensor("w1a", [L, 128, 128], bf16, kind="ExternalInput")
    w1b_in = nc.dram_tensor("w1b", [L, 128, 128], bf16, kind="ExternalInput")
    w1c_in = nc.dram_tensor("w1c", [L, 128, 128], bf16, kind="ExternalInput")
    w1d_in = nc.dram_tensor("w1d", [L, 1, 128], bf16, kind="ExternalInput")
    eb1_in = nc.dram_tensor("eb1", [L, 128, 1], f32, kind="ExternalInput")
    ew2_in = nc.dram_tensor("ew2", [L, 128, 128], bf16, kind="ExternalInput")
    eb2_in = nc.dram_tensor("eb2t4", [L, 1, JW * H], bf16, kind="ExternalInput")
    nw1_in = nc.dram_tensor("nw1", [L, 256, 128], bf16, kind="ExternalInput")
    nb1_in = nc.dram_tensor("nb1", [L, 128, 1], f32, kind="ExternalInput")
    nw2_in = nc.dram_tensor("nw2", [L, 128, 128], bf16, kind="ExternalInput")
    nb2_in = nc.dram_tensor("nb2", [L, 128, 1], f32, kind="ExternalInput")
    linw_in = nc.dram_tensor("linw", [128, 128], f32, kind="ExternalInput")
    linb_in = nc.dram_tensor("linb", [128, 1], f32, kind="ExternalInput")
    cnt_in = nc.dram_tensor("cnt_inv", [G, 1], f32, kind="ExternalInput")
    ones_in = nc.dram_tensor("ones", [1, 512], bf16, kind="ExternalInput")

    out_t = nc.dram_tensor("out", [G, 128], f32, kind="ExternalOutput")

    # ---- internal DRAM state
    hT_st = nc.dram_tensor("hT_st", [128, NPAD], f32)
    ag_in = nc.dram_tensor("ag_in", [NPAD, 128], bf16)
    h_full_ag = nc.dram_tensor("h_full_ag", [NTOT, 128], bf16)
    ar_in = nc.dram_tensor("ar_in", [G, 128], f32)
    ar_out = nc.dram_tensor("ar_out", [G, 128], f32)

    cc_sems = [nc.alloc_semaphore(f"cc_sem_{l}") for l in range(L)]

    gq = [0]

    def next_q():
        q = gq[0]
        gq[0] = (q + 1) % 4
        return q

    for l in range(L):
        gtab = h0_full if l == 0 else h_full_ag
        gather_insts = []
        with tile.TileContext(nc) as tc:
            with (
                tc.tile_pool(name="const", bufs=1) as cpool,
                tc.tile_pool(name="state", bufs=1) as spool,
                tc.tile_pool(name="blk", bufs=2) as bpool,
                tc.tile_pool(name="work", bufs=3) as wpool,
                tc.tile_pool(name="ps1", bufs=2, space="PSUM") as ps1,
                tc.tile_pool(name="ps2", bufs=2, space="PSUM") as ps2,
                tc.tile_pool(name="ps3", bufs=2, space="PSUM") as ps3,
                tc.tile_pool(name="psa", bufs=2, space="PSUM") as psa,
            ):
                # ---------- resident loads
                w1a_t = cpool.tile([128, 128], bf16)
                w1b_t = cpool.tile([128, 128], bf16)
                w1c_t = cpool.tile([128, 128], bf16)
                w1d_t = cpool.tile([1, 128], bf16)
                eb1_t = cpool.tile([128, 1], f32)
                ew2_t = cpool.tile([128, 128], bf16)
                eb2_t = cpool.tile([1, JW * H], bf16)
                nw1a_t = cpool.tile([128, 128], bf16)
                nw1b_t = cpool.tile([128, 128], bf16)
                nb1_t = cpool.tile([128, 1], f32)
                nw2_t = cpool.tile([128, 128], bf16)
                nb2_t = cpool.tile([128, 1], f32)
                ones_t = cpool.tile([1, 512], bf16)
                iota_c = cpool.tile([128, 1], f32)
                idx_t = cpool.tile([128, S_tot * 8], i16)
                nc.sync.dma_start(out=w1a_t[:], in_=w1a_in[l])
                nc.sync.dma_start(out=w1b_t[:], in_=w1b_in[l])
                nc.sync.dma_start(out=w1c_t[:], in_=w1c_in[l])
                nc.sync.dma_start(out=w1d_t[:], in_=w1d_in[l])
                nc.sync.dma_start(out=eb1_t[:], in_=eb1_in[l])
                nc.sync.dma_start(out=ew2_t[:], in_=ew2_in[l])
                nc.sync.dma_start(out=eb2_t[:], in_=eb2_in[l])
                nc.sync.dma_start(out=nw1a_t[:], in_=nw1_in[l, 0:128, :])
                nc.sync.dma_start(out=nw1b_t[:], in_=nw1_in[l, 128:256, :])
                nc.sync.dma_start(out=nb1_t[:], in_=nb1_in[l])
                nc.sync.dma_start(out=nw2_t[:], in_=nw2_in[l])
                nc.sync.dma_start(out=nb2_t[:], in_=nb2_in[l])
                nc.sync.dma_start(out=ones_t[:], in_=ones_in[:])
                nc.sync.dma_start(out=iota_c[:], in_=iota_in[:])
                nc.sync.dma_start(out=idx_t[:], in_=idx_all[:])

                ident_bf = cpool.tile([128, 128], bf16)
                make_identity(nc, ident_bf[:])

                hT = spool.tile([128, NPAD], f32)
                nc.sync.dma_start(out=hT[:], in_=(h0T_in if l == 0 else hT_st)[:])
                hTb = spool.tile([128, NPAD], bf16)
                for t in range(0, NPAD, 512):
                    wd = min(512, NPAD - t)
                    nc.vector.tensor_copy(out=hTb[:, t:t + wd], in_=hT[:, t:t + wd])

                aggb = spool.tile([128, NPAD], bf16)

                # ---------- HW1a = (h @ W1a) per block, node-major (SBUF)
                hw1a_sb = spool.tile([128, NB * 128], bf16)
                for b in range(NB):
                    ps = ps2.tile([128, 128], f32, space="PSUM", tag="ps_wb")
                    nc.tensor.matmul(
                        out=ps[:], lhsT=hTb[:, b * 128:(b + 1) * 128],
                        rhs=w1a_t[:], start=True, stop=True)
                    nc.vector.tensor_copy(
                        out=hw1a_sb[:, b * 128:(b + 1) * 128], in_=ps[:])

                # ---------- edge loop (block-major)
                ji = 0
                for b in range(NB):
                    sb = int(S_b[b])
                    if sb == 0:
                        continue
                    nEb = sb * 128
                    c0 = int(blk_off[b])
                    s0 = int(blk_sub0[b])

                    # block constants: [ea | qs] one DMA; o/d rows one DMA
                    eaqs = bpool.tile([128, 2 * S_MAX * 128], bf16, tag="eaqs")
                    nc.sync.dma_start(
                        out=eaqs[:, :2 * nEb], in_=pack_in[:, c0:c0 + 2 * nEb])
                    od_t = bpool.tile([1, 2 * S_MAX * 128], bf16, tag="od")
                    nc.sync.dma_start(
                        out=od_t[:, :2 * nEb],
                        in_=od_in[:, 2 * s0 * 128:2 * s0 * 128 + 2 * nEb])

                    # gather h[col] feature-major, one call per bank run
                    hcol = bpool.tile([128, 1, S_MAX * 128], bf16, tag="hcol")
                    r0 = 0
                    for k in range(2):
                        nsk = int(S_bk[b, k])
                        src = gtab[BANK_OFF:, :] if k else gtab[:, :]
                        # <=896 idx (7 subtiles) per call: SWDGE ring is
                        # 2048 descs/queue, keep ample headroom
                        o2 = 0
                        while o2 < nsk:
                            cw = min(7, nsk - o2)
                            nidx = cw * 128
                            rr = r0 + o2
                            g_inst = nc.gpsimd.dma_gather(
                                hcol[:, :, rr * 128:rr * 128 + nidx],
                                src,
                                idx_t[:, (s0 + rr) * 8:(s0 + rr + cw) * 8],
                                nidx, nidx, 128,
                                transpose=True, queue_num=next_q())
                            gather_insts.append(g_inst)
                            o2 += cw
                        r0 += nsk

                    # per-block scatter accumulator (f32, SBUF)
                    agg_blk = bpool.tile([128, 128], f32, tag="aggblk")

                    # joints of this block
                    nj = 0
                    while nj < sb:
                        bj, sub0, w = joints[ji]
                        assert bj == b and sub0 == s0 + nj
                        ji += 1
                        nE = w * 128
                        j0 = nj * 128           # col offset within block

                        # qrow one-hot from o row (PE broadcast + DVE is_equal):
                        # qrow[p, e] = (o_e == p)
                        orep = ps1.tile([128, 512], f32, space="PSUM", tag="ps_wa")
                        nc.tensor.matmul(
                            out=orep[:, :nE], lhsT=ones_t[0:1, 0:128],
                            rhs=od_t[0:1, j0:j0 + nE], start=True, stop=True)
                        qrow = wpool.tile([128, 512], bf16, tag="qrow")
                        nc.vector.tensor_scalar(
                            out=qrow[:, :nE], in0=orep[:, :nE],
                            scalar1=iota_c[:, 0:1],
                            scalar2=None, op0=mybir.AluOpType.is_equal)

                        # m1T accumulation [feat, e]
                        m1 = ps2.tile([128, 512], f32, space="PSUM", tag="ps_wb")
                        nc.tensor.matmul(
                            out=m1[:, :nE],
                            lhsT=hw1a_sb[:, b * 128:(b + 1) * 128],
                            rhs=qrow[:, :nE], start=True, stop=False)
                        nc.tensor.matmul(
                            out=m1[:, :nE], lhsT=w1d_t[0:1, :],
                            rhs=od_t[0:1, nEb + j0:nEb + j0 + nE],
                            start=False, stop=False)
                        nc.tensor.matmul(
                            out=m1[:, :nE], lhsT=w1c_t[:],
                            rhs=eaqs[:, j0:j0 + nE], start=False, stop=False)
                        nc.tensor.matmul(
                            out=m1[:, :nE], lhsT=w1b_t[:],
                            rhs=hcol[:, 0, j0:j0 + nE], start=False, stop=True)
                        m1s = wpool.tile([128, 512], bf16, tag="m1s")
                        nc.scalar.activation(
                            out=m1s[:, :nE], in_=m1[:, :nE],
                            func=mybir.ActivationFunctionType.Silu,
                            bias=eb1_t[:, 0:1])

                        # MLP2: merged bias matmul + per-subtile ew2
                        m2 = ps3.tile([128, 512], f32, space="PSUM", tag="ps_wc")
                        for s in range(w):
                            sl = slice(s * 128, (s + 1) * 128)
                            if not EB2_ZERO:
                                nc.tensor.matmul(
                                    out=m2[:, sl], lhsT=ones_t[0:1, 0:128],
                                    rhs=eb2_t[0:1, s * 128:(s + 1) * 128],
                                    start=True, stop=False)
                            nc.tensor.matmul(
                                out=m2[:, sl], lhsT=m1s[:, sl],
                                rhs=ew2_t[:], start=EB2_ZERO, stop=True)
                        m2s = wpool.tile([128, 512], bf16, tag="m2s")
                        nc.scalar.activation(
                            out=m2s[:, :nE], in_=m2[:, :nE],
                            func=mybir.ActivationFunctionType.Silu)

                        # scatter into the block accumulator (qs preloaded)
                        aggp = psa.tile([128, 128], f32, space="PSUM", tag="ps_acc")
                        for s in range(w):
                            sl = slice(s * 128, (s + 1) * 128)
                            nc.tensor.matmul(
                                out=aggp[:], lhsT=m2s[:, sl],
                                rhs=eaqs[:, nEb + j0 + s * 128:
                                         nEb + j0 + (s + 1) * 128],
                                start=(s == 0), stop=(s == w - 1))
                        if nj == 0:
                            nc.vector.tensor_copy(out=agg_blk[:], in_=aggp[:])
                        else:
                            nc.vector.tensor_add(
                                out=agg_blk[:], in0=agg_blk[:], in1=aggp[:])
                        nj += w

                    nc.vector.tensor_copy(
                        out=aggb[:, b * 128:(b + 1) * 128], in_=agg_blk[:])

                assert ji == len(joints)

                # ---------- node MLP + residual
                for t in range(0, NPAD, 512):
                    wd = min(512, NPAD - t)
                    u1 = ps2.tile([128, 512], f32, space="PSUM", tag="ps_wb")
                    nc.tensor.matmul(out=u1[:, :wd], lhsT=nw1a_t[:],
                                     rhs=hTb[:, t:t + wd], start=True, stop=False)
                    nc.tensor.matmul(out=u1[:, :wd], lhsT=nw1b_t[:],
                                     rhs=aggb[:, t:t + wd], start=False, stop=True)
                    u1s = wpool.tile([128, 512], bf16, tag="u1s")
                    nc.scalar.activation(
                        out=u1s[:, :wd], in_=u1[:, :wd],
                        func=mybir.ActivationFunctionType.Silu, bias=nb1_t[:, 0:1])
                    u2 = ps1.tile([128, 512], f32, space="PSUM", tag="ps_wa")
                    nc.tensor.matmul(out=u2[:, :wd], lhsT=nw2_t[:], rhs=u1s[:, :wd],
                                     start=True, stop=True)
                    ub = wpool.tile([128, 512], f32, tag="ub")
                    nc.vector.tensor_scalar(
                        out=ub[:, :wd], in0=u2[:, :wd], scalar1=nb2_t[:, 0:1],
                        scalar2=None, op0=mybir.AluOpType.add)
                    nc.vector.tensor_add(
                        out=hT[:, t:t + wd], in0=hT[:, t:t + wd], in1=ub[:, :wd])

                # new h in bf16 + node-major staging
                for t in range(0, NPAD, 512):
                    wd = min(512, NPAD - t)
                    nc.vector.tensor_copy(out=hTb[:, t:t + wd], in_=hT[:, t:t + wd])
                stage = spool.tile([128, NB * 128], bf16)
                for b in range(NB):
                    tp = ps2.tile([128, 128], bf16, space="PSUM", tag="ps_wb")
                    nc.tensor.transpose(
                        out=tp[:], in_=hTb[:, b * 128:(b + 1) * 128],
                        identity=ident_bf[:])
                    nc.vector.tensor_copy(
                        out=stage[:, b * 128:(b + 1) * 128], in_=tp[:])
                if l < L - 1:
                    nc.sync.dma_start(out=hT_st[:], in_=hT[:])
                    for b in range(NB):
                        nc.sync.dma_start(
                            out=ag_in[b * 128:(b + 1) * 128, :],
                            in_=stage[:, b * 128:(b + 1) * 128])
                else:
                    # pooled partial sums [G, 128]
                    Bq_t = cpool.tile([128, NB * G], bf16)
                    nc.sync.dma_start(out=Bq_t[:], in_=Bq_in[:])
                    sums = ps1.tile([G, 128], f32, space="PSUM", tag="ps_wa")
                    for b in range(NB):
                        nc.tensor.matmul(
                            out=sums[:], lhsT=Bq_t[:, b * G:(b + 1) * G],
                            rhs=stage[:, b * 128:(b + 1) * 128],
                            start=(b == 0), stop=(b == NB - 1))
                    sums_sb = wpool.tile([G, 128], f32, tag="sums_sb")
                    nc.vector.tensor_copy(out=sums_sb[:], in_=sums[:])
                    nc.sync.dma_start(out=ar_in[:], in_=sums_sb[:])

        if l > 0:
            for g in gather_insts:
                g.wait_op(cc_sems[l - 1], 1, "sem-ge", check=False)
        # ---- raw collective between regions
        with (
            nc.Block() as block,
        ):
            if l < L - 1:
                @block.gpsimd
                def _(gpsimd, l=l):
                    gpsimd.collective_compute(
                        "AllGather", mybir.AluOpType.bypass,
                        replica_groups=[list(range(NC))],
                        ins=[ag_in[:, :]], outs=[h_full_ag[:, :]],
                    ).then_inc(cc_sems[l])
            else:
                @block.gpsimd
                def _(gpsimd, l=l):
                    gpsimd.collective_compute(
                        "AllReduce", mybir.AluOpType.add,
                        replica_groups=[list(range(NC))],
                        ins=[ar_in[:, :]], outs=[ar_out[:, :]],
                    ).then_inc(cc_sems[l])
                    gpsimd.wait_ge(cc_sems[l], 1)
        nc.all_engine_barrier()

    # ---- final: pooled mean, relu, linear
    with tile.TileContext(nc) as tc:
        with (
            tc.tile_pool(name="fin", bufs=1) as pool,
            tc.tile_pool(name="finp", bufs=1, space="PSUM") as psum,
        ):
            ar_sb = pool.tile([G, 128], f32)
            nc.sync.dma_start(out=ar_sb[:], in_=ar_out[:])
            cnt_t = pool.tile([G, 1], f32)
            nc.sync.dma_start(out=cnt_t[:], in_=cnt_in[:])
            linw_t = pool.tile([128, 128], f32)
            nc.sync.dma_start(out=linw_t[:], in_=linw_in[:])
            linb_t = pool.tile([128, 1], f32)
            nc.sync.dma_start(out=linb_t[:], in_=linb_in[:])
            ident64 = pool.tile([G, G], f32)
            make_identity(nc, ident64[:])
            ident128 = pool.tile([128, 128], f32)
            make_identity(nc, ident128[:])

            poolR = pool.tile([G, 128], f32)
            nc.scalar.activation(
                out=poolR[:], in_=ar_sb[:],
                func=mybir.ActivationFunctionType.Relu, scale=cnt_t[:, 0:1])
            prT_p = psum.tile([128, G], f32, space="PSUM")
            nc.tensor.transpose(out=prT_p[:], in_=poolR[:], identity=ident64[:])
            prT = pool.tile([128, G], f32)
            nc.vector.tensor_copy(out=prT[:], in_=prT_p[:])
            oT_p = psum.tile([128, G], f32, space="PSUM")
            nc.tensor.matmul(out=oT_p[:], lhsT=linw_t[:], rhs=prT[:],
                             start=True, stop=True)
            oT = pool.tile([128, G], f32)
            nc.vector.tensor_scalar(
                out=oT[:], in0=oT_p[:], scalar1=linb_t[:, 0:1],
                scalar2=None, op0=mybir.AluOpType.add)
            o_p = psum.tile([G, 128], f32, space="PSUM")
            nc.tensor.transpose(out=o_p[:], in_=oT[:], identity=ident128[:])
            o_sb = pool.tile([G, 128], f32)
            nc.vector.tensor_copy(out=o_sb[:], in_=o_p[:])
            nc.sync.dma_start(out=out_t[:], in_=o_sb[:])

    nc.finalize()
    return nc


# ---------------------------------------------------------------- runner
class _SpmdRunner:
    def __init__(self, nc, n_cores=NC):
        install_neuronx_cc_hook()
        self.nc = nc
        self.n_cores = n_cores
        in_names, out_names, out_avals = [], [], []
        pname = nc.partition_id_tensor.name if nc.partition_id_tensor else None
        for alloc in nc.m.functions[0].allocations:
            if not isinstance(alloc, mybir.MemoryLocationSet):
                continue
            name = alloc.memorylocations[0].name
            if alloc.kind == "ExternalInput":
                if name != pname:
                    in_names.append(name)
            elif alloc.kind == "ExternalOutput":
                out_names.append(name)
                out_avals.append(jax.core.ShapedArray(
                    tuple(alloc.tensor_shape), mybir.dt.np(alloc.dtype)))
        self.in_names, self.out_names, self.out_avals = in_names, out_names, out_avals
        n_params, n_outs = len(in_names), len(out_avals)
        all_names = in_names + out_names + ([pname] if pname else [])

        def _body(*args):
            operands = list(args)
            if pname is not None:
                operands.append(partition_id_tensor())
            return tuple(_bass_exec_p.bind(
                *operands,
                out_avals=tuple(out_avals), in_names=tuple(all_names),
                out_names=tuple(out_names), lowering_input_output_aliases=(),
                sim_require_finite=True, sim_require_nnan=True, nc=nc))

        devices = jax.devices()[:n_cores]
        self.mesh = Mesh(np.asarray(devices), ("core",))
        specs = (PartitionSpec("core"),) * (n_params + n_outs)
        self.fn = jax.jit(
            shard_map(_body, mesh=self.mesh, in_specs=specs,
                      out_specs=(PartitionSpec("core"),) * n_outs,
                      check_rep=False),
            keep_unused=True)
        self._zero_outs = [
            np.zeros((n_cores * a.shape[0], *a.shape[1:]), a.dtype)
            for a in out_avals]

    def stage(self, in_maps):
        sharding = jax.sharding.NamedSharding(self.mesh, PartitionSpec("core"))
        staged = []
        for name in self.in_names:
            arrs = [np.asarray(m[name]) for m in in_maps]
            staged.append(jax.device_put(np.concatenate(arrs, 0), sharding))
        for z in self._zero_outs:
            staged.append(jax.device_put(z, sharding))
        return staged

    def run(self, staged):
        outs = self.fn(*staged)
        jax.block_until_ready(outs)
        return outs

    def result_core0(self, outs, name):
        i = self.out_names.index(name)
        a = np.asarray(outs[i])
        return a.reshape(self.n_cores, *self.out_avals[i].shape)[0]


_CACHE = {}


def kernel(**inputs) -> np.ndarray:
    _install_birfix()
    per_core, struct = _preprocess(**inputs)
    key = (struct["S_tot"], struct["T_joint"], struct["eb2_zero"])
    if key not in _CACHE:
        nc = _build_program(struct)
        _CACHE[key] = _SpmdRunner(nc)
    runner = _CACHE[key]
    staged = runner.stage(per_core)
    outs = runner.run(staged)
    res = runner.result_core0(outs, "out")
    _CACHE["last"] = (runner, staged)
    return np.asarray(res, np.float32)
